# revision 15
# baseline (speedup 1.0000x reference)
"""Trainium2 Bass kernel for 16-head attention (B=2, N=2048, D=1024).

Sharding: 8 cores = 2 batches x 4 head-groups (4 heads each).
Each core computes q/k/v projections for its heads, per-head softmax
attention, and a partial output projection. Host sums the 4 partials
per batch and adds the bias.

Device layout notes:
  - x is passed pre-transposed (xT [D, N]) so it can serve as the
    moving operand of the projection matmuls directly.
  - scores are computed transposed (S^T[n2, n1] = k^T.T @ q^T) so the
    exp(S^T) tiles feed attn@v as the moving operand with no transpose.
  - attn@v uses v with an appended ones-column: out rows 0-63 give
    v^T @ expS^T (= attnout^T), row 64 gives the softmax denominator.
"""

import os
import sys

import numpy as np

sys.path.insert(0, "/opt/trn_rl_repo")

B, N, D = 2, 2048, 1024
NUM_HEADS = 16
HD = 64
N_CORES = 8
HEADS_PER_CORE = 4  # 16 heads / (8 cores / 2 batches)
HCOLS = HEADS_PER_CORE * HD  # 256
SCALE = HD ** -0.5  # 0.125

P = 128
KC = D // P  # 8 contraction chunks
NT = N // P  # 16 sequence tiles of 128
NQ = N // 512  # 4 sequence chunks of 512

# Set True to route the odd-head division output through an SBUF->SBUF DMA
# instead of a partition-shifted DVE write (fallback if DVE rejects it).
DVE_PARTITION_SHIFT_OK = True


def build_program(mm_dtype_name: str = "float32r", loop_n: int | None = None):
    """loop_n: if set, wrap the whole kernel body in a hardware For_i loop
    executing it loop_n times (used to measure per-iteration HW time by slope
    between two loop counts)."""
    import contextlib

    import concourse.bass as bass
    import concourse.tile as tile
    from concourse import bacc, mybir

    mmdt = getattr(mybir.dt, mm_dtype_name)
    f32 = mybir.dt.float32

    def rb(ap):
        # bitcast a DRAM f32 AP so DMA writes into an mmdt-typed tile
        return ap.bitcast(mmdt) if mmdt != f32 else ap

    nc = bacc.Bacc()

    xT_d = nc.dram_tensor("xT", [D, N], f32, kind="ExternalInput")
    wq_d = nc.dram_tensor("wq", [D, HCOLS], f32, kind="ExternalInput")
    wk_d = nc.dram_tensor("wk", [D, HCOLS], f32, kind="ExternalInput")
    wv_d = nc.dram_tensor("wv", [D, HCOLS], f32, kind="ExternalInput")
    wp_d = nc.dram_tensor("wp", [HCOLS, D], f32, kind="ExternalInput")
    out_d = nc.dram_tensor("outp", [N, D], f32, kind="ExternalOutput")
    lbuf_d = nc.dram_tensor("lbuf", [HEADS_PER_CORE, N], f32)

    with tile.TileContext(nc) as tc:
        with (
            tc.tile_pool(name="persist", bufs=1) as persist,
            tc.tile_pool(name="work", bufs=2) as work,
            tc.tile_pool(name="psum", bufs=1, space="PSUM") as psum,
            tc.For_i(0, loop_n, 1, hint_engines=(mybir.EngineType.PE,))
            if loop_n is not None
            else contextlib.nullcontext(),
        ):
            # ---- persistent SBUF tiles ----
            xt = persist.tile([P, KC, N], mmdt)  # x^T chunks; reused as aoT later
            wq = persist.tile([P, KC, HCOLS], mmdt)
            wk = persist.tile([P, KC, HCOLS], mmdt)
            wv = persist.tile([P, KC, HCOLS], mmdt)
            wp = persist.tile([P, 2, D], mmdt)
            qT = persist.tile([P, 2, N], mmdt)
            kT = persist.tile([P, 2, N], mmdt)
            vext = persist.tile([P, HEADS_PER_CORE, NT * 65], mmdt)
            lb = persist.tile([65, N], f32)  # row 64: 1/l; rows 0-63: broadcast
            zbias = persist.tile([P, 1], f32)
            ones64 = persist.tile([P, 64], f32)

            nc.vector.memset(zbias, 0.0)
            # ones columns for the denominator trick (f32 memset + rounding
            # copy into the f32r tile; direct f32r memset fails the ISA check)
            nc.vector.memset(ones64, 1.0)
            nc.vector.tensor_copy(
                vext.rearrange("p h (t c) -> p h t c", c=65)[:, :, :, 64],
                ones64.rearrange("p (h t) -> p h t", h=HEADS_PER_CORE),
            )

            # ---- input DMAs ----
            for kc in range(KC):
                nc.sync.dma_start(out=xt[:, kc, :], in_=rb(xT_d[kc * P : (kc + 1) * P, :]))
            nc.sync.dma_start(out=wq, in_=rb(wq_d.rearrange("(c p) f -> p c f", p=P)))
            nc.sync.dma_start(out=wk, in_=rb(wk_d.rearrange("(c p) f -> p c f", p=P)))
            nc.sync.dma_start(out=wv, in_=rb(wv_d.rearrange("(c p) f -> p c f", p=P)))
            nc.sync.dma_start(out=wp, in_=rb(wp_d.rearrange("(c p) f -> p c f", p=P)))

            # ---- phase 1a: q^T, k^T = (x @ wq/wk)^T  [256, N] each ----
            for dst, w in ((qT, wq), (kT, wk)):
                for m in range(2):
                    ps = psum.tile([P, N], f32, tag="big", bufs=1, name="ps_qk")
                    for kc in range(KC):
                        for n1c in range(NQ):
                            nc.tensor.matmul(
                                ps[:, n1c * 512 : (n1c + 1) * 512],
                                w[:, kc, m * P : (m + 1) * P],
                                xt[:, kc, n1c * 512 : (n1c + 1) * 512],
                                start=(kc == 0),
                                stop=(kc == KC - 1),
                            )
                    nc.vector.tensor_copy(dst[:, m, :], ps)

            # ---- phase 1b: v natural [N, 256] -> vext per head ----
            for nt in range(NT):
                psv = psum.tile([P, 1024], f32, tag="sc", bufs=2, name="ps_v")
                for kc in range(KC):
                    nc.tensor.matmul(
                        psv[:, :HCOLS],
                        xt[:, kc, nt * P : (nt + 1) * P],
                        wv[:, kc, :],
                        start=(kc == 0),
                        stop=(kc == KC - 1),
                    )
                nc.vector.tensor_copy(
                    vext[:, :, nt * 65 : nt * 65 + 64],
                    psv[:, :HCOLS].rearrange("p (h d) -> p h d", h=HEADS_PER_CORE),
                )

            # ---- phase 2: per-head attention ----
            for h in range(HEADS_PER_CORE):
                hp, a = divmod(h, 2)
                ps_o = psum.tile([65, N], f32, tag="big", bufs=1, name="ps_o")
                for t2 in range(NT):
                    expS = work.tile([P, N], mmdt, tag="expS", bufs=2, name="expS")
                    for half in range(2):
                        pssc = psum.tile([P, 1024], f32, tag="sc", bufs=2, name="ps_sc")
                        for q in range(2):
                            n1c = 2 * half + q
                            nc.tensor.matmul(
                                pssc[:, q * 512 : (q + 1) * 512],
                                kT[64 * a : 64 * a + 64, hp, t2 * P : (t2 + 1) * P],
                                qT[64 * a : 64 * a + 64, hp, n1c * 512 : (n1c + 1) * 512],
                                start=True,
                                stop=True,
                            )
                        nc.scalar.activation(
                            expS[:, half * 1024 : (half + 1) * 1024],
                            pssc,
                            bass.mybir.ActivationFunctionType.Exp,
                            bias=zbias,
                            scale=SCALE,
                        )
                    for n1c in range(NQ):
                        nc.tensor.matmul(
                            ps_o[:, n1c * 512 : (n1c + 1) * 512],
                            vext[:, h, t2 * 65 : t2 * 65 + 65],
                            expS[:, n1c * 512 : (n1c + 1) * 512],
                            start=(t2 == 0),
                            stop=(t2 == NT - 1),
                        )
                # softmax denominator: divide rows 0-63 by row 64
                nc.vector.reciprocal(lb[64:65, :], ps_o[64:65, :])
                # partition-broadcast via DRAM bounce (step-0 partition AP
                # is only legal with a DRAM source)
                nc.gpsimd.dma_start(out=lbuf_d[h, :], in_=lb[64:65, :])
                src = lbuf_d[h, :]
                bc = bass.AP(
                    tensor=src.tensor,
                    offset=src.offset,
                    ap=[[0, 64]] + [list(d) for d in src.ap],
                )
                nc.gpsimd.dma_start(out=lb[0:64, :], in_=bc)
                # attnout^T for this head -> aoT rows [64a : 64a+64] (aliases xt)
                if a == 0 or DVE_PARTITION_SHIFT_OK:
                    nc.vector.tensor_mul(
                        xt[64 * a : 64 * a + 64, hp, :], ps_o[0:64, :], lb[0:64, :]
                    )
                else:
                    tmp = work.tile([64, N], f32, tag="aotmp", bufs=2, name="aotmp")
                    nc.vector.tensor_mul(tmp, ps_o[0:64, :], lb[0:64, :])
                    nc.sync.dma_start(out=xt[64:128, hp, :], in_=tmp)

            # ---- phase 3: partial projection out = attnout @ wp ----
            for nt in range(NT):
                osb = work.tile([P, D], f32, tag="osb", bufs=2, name="osb")
                for jc in range(2):
                    pj = psum.tile([P, 1024], f32, tag="sc", bufs=2, name="ps_pj")
                    for dk in range(2):
                        nc.tensor.matmul(
                            pj[:, :512],
                            xt[:, dk, nt * P : (nt + 1) * P],
                            wp[:, dk, jc * 512 : (jc + 1) * 512],
                            start=(dk == 0),
                            stop=(dk == 1),
                        )
                    nc.vector.tensor_copy(osb[:, jc * 512 : (jc + 1) * 512], pj[:, :512])
                nc.sync.dma_start(out=out_d[nt * P : (nt + 1) * P, :], in_=osb)

    nc.finalize()
    return nc


def make_in_maps(x, w_qk, w_v, w_proj):
    """Slice + transpose full inputs into per-core input dicts."""
    in_maps = []
    xTb = [np.ascontiguousarray(x[b].T) for b in range(B)]
    for c in range(N_CORES):
        b, g = divmod(c, N_CORES // B)
        h0 = g * HCOLS
        in_maps.append(
            {
                "xT": xTb[b],
                "wq": np.ascontiguousarray(w_qk[:, h0 : h0 + HCOLS]),
                "wk": np.ascontiguousarray(w_qk[:, D + h0 : D + h0 + HCOLS]),
                "wv": np.ascontiguousarray(w_v[:, h0 : h0 + HCOLS]),
                "wp": np.ascontiguousarray(w_proj[h0 : h0 + HCOLS, :]),
            }
        )
    return in_maps


def combine_results(results, b_proj):
    gpb = N_CORES // B
    out = np.empty((B, N, D), dtype=np.float32)
    for b in range(B):
        acc = results[b * gpb]["outp"].astype(np.float32)
        for g in range(1, gpb):
            acc = acc + results[b * gpb + g]["outp"]
        out[b] = acc + b_proj[None, :]
    return out


_CACHE = {}


def _pjrt_runner(nc):
    """Build a sharded 8-core single-exec runner for `nc` (mimics
    bass2jax.run_bass_via_pjrt). Returns run_fn(in_maps) -> per-core out dicts,
    and timed_fn(in_maps, reps) -> best wall seconds for one execution."""
    import time

    import jax
    from jax.experimental.shard_map import shard_map
    from jax.sharding import Mesh, NamedSharding, PartitionSpec

    from concourse import bass2jax, mybir

    bass2jax.install_neuronx_cc_hook()

    partition_name = nc.partition_id_tensor.name if nc.partition_id_tensor else None

    in_names, out_names, out_avals, zero_outs = [], [], [], []
    for alloc in nc.m.functions[0].allocations:
        if not isinstance(alloc, mybir.MemoryLocationSet):
            continue
        name = alloc.memorylocations[0].name
        if alloc.kind == "ExternalInput":
            if name != partition_name:
                in_names.append(name)
        elif alloc.kind == "ExternalOutput":
            out_names.append(name)
            shape = tuple(alloc.tensor_shape)
            dtype = mybir.dt.np(alloc.dtype)
            out_avals.append(jax.core.ShapedArray(shape, dtype))
            zero_outs.append(np.zeros(shape, dtype))
    n_params = len(in_names)
    n_outs = len(out_names)
    all_names = in_names + out_names
    if partition_name is not None:
        all_names = all_names + [partition_name]

    def _body(*args):
        operands = list(args)
        if partition_name is not None:
            operands.append(bass2jax.partition_id_tensor())
        return tuple(
            bass2jax._bass_exec_p.bind(
                *operands,
                out_avals=tuple(out_avals),
                in_names=tuple(all_names),
                out_names=tuple(out_names),
                lowering_input_output_aliases=(),
                sim_require_finite=True,
                sim_require_nnan=True,
                nc=nc,
            )
        )

    devices = jax.devices()[:N_CORES]
    mesh = Mesh(np.asarray(devices), ("core",))
    spec = NamedSharding(mesh, PartitionSpec("core"))

    fn = jax.jit(
        shard_map(
            _body,
            mesh=mesh,
            in_specs=(PartitionSpec("core"),) * (n_params + n_outs),
            out_specs=(PartitionSpec("core"),) * n_outs,
            check_rep=False,
        ),
        donate_argnums=tuple(range(n_params, n_params + n_outs)),
        keep_unused=True,
    )

    def _concat_inputs(in_maps):
        per_core = [[np.asarray(m[name]) for name in in_names] for m in in_maps]
        return [
            np.concatenate([per_core[c][i] for c in range(N_CORES)], axis=0)
            for i in range(n_params)
        ]

    def _zeros():
        return [
            jax.device_put(np.zeros((N_CORES * z.shape[0], *z.shape[1:]), z.dtype), spec)
            for z in zero_outs
        ]

    def run_fn(in_maps):
        ins = [jax.device_put(a, spec) for a in _concat_inputs(in_maps)]
        outs = fn(*ins, *_zeros())
        outs = [np.asarray(o) for o in outs]
        return [
            {
                name: outs[i].reshape(N_CORES, *out_avals[i].shape)[c]
                for i, name in enumerate(out_names)
            }
            for c in range(N_CORES)
        ]

    def timed_fn(in_maps, reps=7):
        ins = [jax.device_put(a, spec) for a in _concat_inputs(in_maps)]
        o = fn(*ins, *_zeros())  # warm-up (compiles)
        jax.block_until_ready(o)
        best = float("inf")
        for _ in range(reps):
            z = _zeros()
            jax.block_until_ready(z)
            t0 = time.perf_counter()
            o = fn(*ins, *z)
            jax.block_until_ready(o)
            best = min(best, time.perf_counter() - t0)
        return best

    return run_fn, timed_fn


LOOP_A, LOOP_B = 4, 20


def measure_hw_time(in_maps, reps=7):
    """Per-iteration HW time via slope between two For_i loop counts."""
    times = {}
    for ln in (LOOP_A, LOOP_B):
        key = ("loop_nc", ln)
        if key not in _CACHE:
            _CACHE[key] = _pjrt_runner(build_program(loop_n=ln))
        _, timed_fn = _CACHE[key]
        times[ln] = timed_fn(in_maps, reps=reps)
    per_iter = (times[LOOP_B] - times[LOOP_A]) / (LOOP_B - LOOP_A)
    return per_iter * 1e9, times


def get_runner():
    if "runner" not in _CACHE:
        _CACHE["runner"] = _pjrt_runner(build_program())
    return _CACHE["runner"]


def run_on_hw(x, w_qk, w_v, w_proj, b_proj):
    run_fn, _ = get_runner()
    in_maps = make_in_maps(x, w_qk, w_v, w_proj)
    results = run_fn(in_maps)
    return combine_results(results, b_proj)


def kernel(x, w_qk, w_v, w_proj, b_proj):
    x = np.asarray(x, dtype=np.float32)
    w_qk = np.asarray(w_qk, dtype=np.float32)
    w_v = np.asarray(w_v, dtype=np.float32)
    w_proj = np.asarray(w_proj, dtype=np.float32)
    b_proj = np.asarray(b_proj, dtype=np.float32)
    return run_on_hw(x, w_qk, w_v, w_proj, b_proj)
